# revision 30
# baseline (speedup 1.0000x reference)
"""MoE router gate (group-limited top-k) on 8 Trainium2 NeuronCores.

reference math (per token row of x [T=16384, D=4096], W [E=256, D]):
  logits = x @ W.T                      [T, 256]
  scores = softmax(logits)
  group (8 groups of 32) scores = max of scores per group
  keep top-4 groups, mask rest to -inf
  top-8 experts over masked scores -> indices
  weights = gathered softmax scores, renormalized over the 8 (+1e-9 in denom)

Sharding: data-parallel over tokens, 2048 tokens/core.

Split-precision fp16 matmul (PE fp32 matmul = 4 cyc/row; fp16 = 1 cyc/row):
  x = a + b/2048 + eps_x   (a = fp16(x), b = fp16((x-a)*2048), |eps| ~ 2^-22|x|)
  w = c + d/2048 + eps_w   (same split for W.T)
  x.w = a.c + (a.d + b.c)/2048 + O(2^-22)
Three fp16 passes instead of four fp32 passes, all products exact in fp32
PSUM; dropped b.d/2048^2 term ~1e-7 on logits ~N(0,1) — well inside fp32
accumulation-order noise vs the CPU reference. All fp16 operands are kept
in the normal range (host flushes subnormal hi/lo parts; the 2048 scaling
keeps lo parts normal) so PE subnormal handling can't perturb results.

x is transposed on the host (like W already was) so the contraction dim d
lands on SBUF partitions: no PE transposes at all.

Per-core program: 4 sweeps x 4 token tiles. Each sweep holds 4 PSUM banks
[128t, 512] (cols 0:256 accumulate a.c, 256:512 accumulate a.d' + b'.c),
accumulating over 32 k-chunks: per (tile, chunk) one N=512 matmul
(stationary a, moving [c|d']) + one N=256 matmul (stationary b', moving c).
Consecutive sweeps alternate PSUM bank halves (pool bufs=8) so the DVE
combine (logits = P1 + P2/2048) and selection drain one sweep while the PE
streams the next. Selection math as before: group-limited top-k on raw
logits (softmax is monotone per row), weights = exp renormalized via ACT.
"""

import numpy as np

from concourse import bass, mybir
from concourse.bacc import Bacc
from concourse.tile import TileContext
from concourse.bass_utils import run_bass_kernel_spmd

TOKENS = 16384
DIM = 4096
E = 256
TOPK = 8
G = 8
GSZ = E // G  # 32
NL = 4  # groups kept
N_CORES = 8
TPC = TOKENS // N_CORES  # 2048 tokens per core
KC = DIM // 128  # 32 contraction chunks
SW = 4  # token tiles per sweep
NSW = TPC // (SW * 128)  # 4 sweeps
SWT = SW * 128  # 512 tokens per sweep
NEG_BIG = -1.0e30
SCALE = 2048.0  # 2^11 scaling of the lo split parts
FP16_MIN_NORMAL = 2.0 ** -14
REPEAT = 1  # bench-only: replicate the sweep loop on device

_CACHE = {}


def _build_program(repeat=1):
    nc = Bacc()
    # xz packs the two fp16 split halves sweep-contiguously: for a sweep of
    # swn tiles starting at token t0, cols [2*t0, 2*t0+swt) hold xa (hi) and
    # [2*t0+swt, 2*t0+2*swt) hold xb (lo*2048) — one full-speed DMA per
    # (chunk, sweep) regardless of sweep size.
    xz_ext = nc.declare_dram_parameter(
        "xz", [DIM, 2 * TPC], mybir.dt.float16, isOutput=False
    )
    cd_ext = nc.declare_dram_parameter("cd", [DIM, 2 * E], mybir.dt.float16, isOutput=False)
    w_out = nc.declare_dram_parameter(
        "weights", [TPC, TOPK], mybir.dt.float32, isOutput=True
    )
    i_out = nc.declare_dram_parameter(
        "indices", [TPC, TOPK], mybir.dt.int32, isOutput=True
    )

    f32 = mybir.dt.float32
    f16 = mybir.dt.float16

    with TileContext(nc) as tc:
        with (
            tc.tile_pool(name="const", bufs=1) as const_pool,
            tc.tile_pool(name="xab", bufs=8) as xab_pool,
            tc.tile_pool(name="ps", bufs=2, space="PSUM") as ps_pool,
            tc.tile_pool(name="lg", bufs=2) as lg_pool,
            tc.tile_pool(name="mid", bufs=4) as mid_pool,
            tc.tile_pool(name="small", bufs=4) as small_pool,
        ):
            # W.T split, resident in SBUF: per chunk k a [128, 512] tile,
            # cols 0:256 = c (hi), 256:512 = d' (lo * 2048). Per-chunk DMAs
            # on the otherwise-idle Pool software DGE: cd[k] lands at ~500ns*k
            # while the PE consumes chunks at ~1300ns*k, so matmuls never
            # wait on the weights and the x-load queues stay clean.
            cds = []
            for k in range(KC):
                t = const_pool.tile([128, 2 * E], f16, tag=f"cd{k}", name=f"cd{k}")
                nc.gpsimd.dma_start(out=t[:], in_=cd_ext[k * 128 : (k + 1) * 128, :])
                cds.append(t)

            # tile counts per sweep: tail sweeps are small so the final
            # drain (combine + selection of the last sweep) is short
            sweep_sizes = [4] * (NSW - 1) + [2, 2]
            assert sum(sweep_sizes) == TPC // 128  # must match SWEEP_SIZES

            for _r in range(repeat):
                t0 = 0
                for sw, swn in enumerate(sweep_sizes):
                    swt = swn * 128
                    ps = [
                        ps_pool.tile([128, 2 * E], f32, tag=f"ps{ti}", name=f"ps{ti}")
                        for ti in range(swn)
                    ]
                    for k in range(KC):
                        xab = xab_pool.tile([128, 2 * swt], f16, tag=f"xab{swn}")
                        nc.sync.dma_start(
                            out=xab[:],
                            in_=xz_ext[
                                k * 128 : (k + 1) * 128, 2 * t0 : 2 * t0 + 2 * swt
                            ],
                        )
                        for ti in range(swn):
                            # P[:,0:256] += a.c ; P[:,256:512] += a.d'
                            nc.tensor.matmul(
                                ps[ti][:],
                                lhsT=xab[:, ti * 128 : (ti + 1) * 128],
                                rhs=cds[k][:],
                                start=(k == 0),
                                stop=False,
                                skip_group_check=True,
                            )
                            # P[:,256:512] += b'.c
                            nc.tensor.matmul(
                                ps[ti][:, E : 2 * E],
                                lhsT=xab[:, swt + ti * 128 : swt + (ti + 1) * 128],
                                rhs=cds[k][:, 0:E],
                                start=False,
                                stop=(k == KC - 1),
                                skip_group_check=True,
                            )

                    # Drain: combines first (frees PSUM banks for the next
                    # sweep ahead of the PE), then selection per tile.
                    # combine: logits = P1 + P2/2048. HW allows only one
                    # PSUM input per instruction, so ACT copies P1 to SBUF
                    # and DVE adds the scaled P2 on top (in place).
                    lgs = []
                    for ti in range(swn):
                        logits = lg_pool.tile([128, E], f32, tag=f"lg{ti}")
                        nc.scalar.copy(logits[:], ps[ti][:, 0:E])
                        nc.vector.scalar_tensor_tensor(
                            logits[:],
                            in0=ps[ti][:, E : 2 * E],
                            scalar=1.0 / SCALE,
                            in1=logits[:],
                            op0=mybir.AluOpType.mult,
                            op1=mybir.AluOpType.add,
                        )
                        lgs.append(logits)

                    for ti in range(swn):
                        logits = lgs[ti]
                        r0 = t0 + ti * 128

                        # ---- selection on raw logits ----
                        gs = small_pool.tile([128, G], f32, tag="gs")
                        nc.vector.tensor_reduce(
                            gs[:],
                            logits[:].rearrange("p (g e) -> p g e", g=G),
                            axis=mybir.AxisListType.X,
                            op=mybir.AluOpType.max,
                        )
                        gsort = small_pool.tile([128, 8], f32, tag="gsort")
                        nc.vector.max(out=gsort[:], in_=gs[:])
                        # softmax shift M = row max = max group score (the
                        # global-max expert's group always survives masking),
                        # so the full-row exp sum Z can start on ACT before
                        # the DVE top-8 path finishes.
                        negm = small_pool.tile([128, 1], f32, tag="negm")
                        nc.gpsimd.tensor_scalar_mul(negm[:], gsort[:, 0:1], -1.0)
                        scr = mid_pool.tile([128, E], f32, tag="scr")
                        zfull = small_pool.tile([128, 1], f32, tag="zfull")
                        nc.scalar.activation(
                            scr[:],
                            logits[:],
                            mybir.ActivationFunctionType.Exp,
                            bias=negm[:],
                            accum_out=zfull[:],
                        )
                        # bias per group: (gs < 4th-largest) * -1e30
                        bias8 = small_pool.tile([128, G], f32, tag="bias8")
                        nc.gpsimd.tensor_scalar(
                            bias8[:],
                            gs[:],
                            gsort[:, NL - 1 : NL],
                            NEG_BIG,
                            op0=mybir.AluOpType.is_lt,
                            op1=mybir.AluOpType.mult,
                        )
                        masked = mid_pool.tile([128, E], f32, tag="masked")
                        for g in range(G):
                            nc.gpsimd.tensor_scalar_add(
                                masked[:, g * GSZ : (g + 1) * GSZ],
                                logits[:, g * GSZ : (g + 1) * GSZ],
                                bias8[:, g : g + 1],
                            )
                        vals8 = small_pool.tile([128, 8], f32, tag="vals8")
                        nc.vector.max(out=vals8[:], in_=masked[:])
                        idx8 = small_pool.tile([128, 8], mybir.dt.uint32, tag="idx8")
                        nc.vector.max_index(
                            out=idx8[:], in_max=vals8[:], in_values=masked[:]
                        )

                        # ---- weights: e_k / (S + 1e-9 * Z), shifted by M
                        e8 = small_pool.tile([128, 8], f32, tag="e8")
                        s8 = small_pool.tile([128, 1], f32, tag="s8")
                        nc.scalar.activation(
                            e8[:],
                            vals8[:],
                            mybir.ActivationFunctionType.Exp,
                            bias=negm[:],
                            accum_out=s8[:],
                        )
                        den = small_pool.tile([128, 1], f32, tag="den")
                        nc.gpsimd.tensor_scalar(
                            den[:],
                            zfull[:],
                            1.0e-9,
                            None,
                            op0=mybir.AluOpType.mult,
                        )
                        nc.gpsimd.tensor_add(den[:], den[:], s8[:])
                        rcp = small_pool.tile([128, 1], f32, tag="rcp")
                        nc.vector.reciprocal(rcp[:], den[:])
                        w8 = small_pool.tile([128, 8], f32, tag="w8")
                        nc.gpsimd.tensor_scalar_mul(w8[:], e8[:], rcp[:])

                        # outputs bypass SP's x-load queue: weights via the
                        # ACT hwdge, indices via the Pool software DGE (which
                        # also casts uint32->int32; values <= 255, same bits)
                        nc.scalar.dma_start(out=w_out[r0 : r0 + 128, :], in_=w8[:])
                        nc.gpsimd.dma_start(out=i_out[r0 : r0 + 128, :], in_=idx8[:])
                    t0 += swt
    return nc


def _split_fp16(v: np.ndarray):
    """v (f32) -> (hi f16, lo f16) with v ~ hi + lo/2048, subnormals flushed."""
    hi = v.astype(np.float16)
    hi[np.abs(hi) < FP16_MIN_NORMAL] = np.float16(0)
    lo = ((v - hi.astype(np.float32)) * SCALE).astype(np.float16)
    lo[np.abs(lo) < FP16_MIN_NORMAL] = np.float16(0)
    return hi, lo


def _weight_cd(weight: np.ndarray) -> np.ndarray:
    wt = np.ascontiguousarray(weight.T, dtype=np.float32)  # [DIM, E]
    c, d = _split_fp16(wt)
    return np.ascontiguousarray(np.concatenate([c, d], axis=1))  # [DIM, 2E]


SWEEP_SIZES = [4] * (NSW - 1) + [2, 2]


def _pack_xz(xa: np.ndarray, xb: np.ndarray) -> np.ndarray:
    """[DIM, n*TPC] hi/lo -> sweep-packed [DIM, 2*n*TPC] (see xz layout)."""
    d, nt = xa.shape
    z = np.empty((d, 2 * nt), dtype=np.float16)
    for c0 in range(0, nt, TPC):
        t0 = 0
        for swn in SWEEP_SIZES:
            swt = swn * 128
            a0 = 2 * (c0 + t0)
            z[:, a0 : a0 + swt] = xa[:, c0 + t0 : c0 + t0 + swt]
            z[:, a0 + swt : a0 + 2 * swt] = xb[:, c0 + t0 : c0 + t0 + swt]
            t0 += swt
    return z


def make_core_inputs(x_core: np.ndarray, weight: np.ndarray) -> dict:
    """Per-core input map for one core's token slice (used by sim_run.py)."""
    xt = np.ascontiguousarray(x_core.T, dtype=np.float32)  # [DIM, TPC]
    xa, xb = _split_fp16(xt)
    return {"xz": _pack_xz(xa, xb), "cd": _weight_cd(weight)}


def get_program(repeat=1):
    key = ("nc", repeat)
    if key not in _CACHE:
        nc = _build_program(repeat)
        # Bacc defers register allocation + wait-splitting to finalize();
        # the PJRT path serializes the module as-is, so lower it now.
        nc.finalize()
        _CACHE[key] = nc
    return _CACHE[key]


def kernel(x: np.ndarray, weight: np.ndarray, **run_kwargs):
    xt = np.ascontiguousarray(np.asarray(x, dtype=np.float32).T)  # [DIM, TOKENS]
    xa, xb = _split_fp16(xt)
    xz = _pack_xz(xa, xb)  # [DIM, 2*TOKENS]
    cd = _weight_cd(np.asarray(weight))
    nc = get_program()
    in_maps = [
        {
            "xz": np.ascontiguousarray(xz[:, c * 2 * TPC : (c + 1) * 2 * TPC]),
            "cd": cd,
        }
        for c in range(N_CORES)
    ]
    res = run_bass_kernel_spmd(nc, in_maps, list(range(N_CORES)), **run_kwargs)
    weights = np.concatenate([res.results[c]["weights"] for c in range(N_CORES)], axis=0)
    indices = np.concatenate([res.results[c]["indices"] for c in range(N_CORES)], axis=0)
    _CACHE["last_results"] = res
    return weights.astype(np.float32), indices.astype(np.int32)
